# revision 7
# baseline (speedup 1.0000x reference)
"""NestedMLP MoE-routed kernel for 8 TRN2 NeuronCores.

Strategy:
  - Host routes tokens by expert (argsort of expert_mask), splits each
    expert's tokens across the 8 cores (data-parallel), pads each
    per-core expert group to a common capacity so all cores run one SPMD
    program.
  - Activations are kept feature-major ("transposed", [feature, token])
    so both matmuls are natural lhsT.T @ rhs with the contraction dim on
    partitions, and the per-feature biases are per-partition (fusable
    into the ACT/DVE PSUM eviction).
  - Weights/activations are bf16 (f32 PSUM accumulation); biases and the
    output stay f32.
  - Per expert e (shift = 3-e): d_in = 1024>>shift, d_hid = 4*d_in,
    d_out = 1024>>shift, using the nested weight slices
    w1[:d_hid,:d_in], w2[:d_out,:d_hid].
"""

import math
import sys
import types

sys.path.insert(0, "/opt/trn_rl_repo")

import ml_dtypes
import numpy as np

P = 128
E = 4
D = 1024
H = 4096
OUT = 1024
NCORES = 8
MLP_RATIO = 4

BF16 = ml_dtypes.bfloat16

# (d_in, d_hid, d_out) per expert
DIMS = [((D >> (E - 1 - e)), (D >> (E - 1 - e)) * MLP_RATIO, (OUT >> (E - 1 - e))) for e in range(E)]
# chunk width (token columns per matmul pass) per expert; expert 3 uses 256
# so its full hidden tile ([128, 32, cw] bf16) fits in SBUF next to the
# resident weights.
CHUNK_W = [512, 512, 512, 256]


def _round_up(v, m):
    return ((v + m - 1) // m) * m


def _tile_fmajor(a2d):
    """[F, C] -> [128, F//128, C] with row f = po*128 + pi."""
    f, c = a2d.shape
    return np.ascontiguousarray(a2d.reshape(f // P, P, c).transpose(1, 0, 2))


def _nested_extents(full, per_expert):
    """Column extents [(lo, hi)] that successive experts add. per_expert:
    list of high-watermarks per expert."""
    exts = []
    prev = 0
    for hi in per_expert:
        if hi > prev:
            exts.append((prev, hi))
            prev = hi
    assert prev == full
    return exts


def _build_graph(caps):
    """Build the SPMD Bass graph for per-core per-expert capacities `caps`."""
    import concourse.mybir as mybir
    import concourse.tile as tile
    from concourse import bacc

    f32 = mybir.dt.float32
    bf16 = mybir.dt.bfloat16
    Gelu = mybir.ActivationFunctionType.Gelu

    ctot = sum(caps)
    offs = np.concatenate([[0], np.cumsum(caps)]).astype(int)

    nc = bacc.Bacc(None, target_bir_lowering=False, debug=False)
    xt_d = nc.declare_dram_parameter("xt", [P, D // P, ctot], bf16, isOutput=False)
    w1_d = nc.declare_dram_parameter("w1t", [P, D // P, H], bf16, isOutput=False)
    w2_d = nc.declare_dram_parameter("w2t", [P, H // P, OUT], bf16, isOutput=False)
    b1_d = nc.declare_dram_parameter("b1t", [P, H // P], f32, isOutput=False)
    b2_d = nc.declare_dram_parameter("b2t", [P, OUT // P], f32, isOutput=False)
    y_d = nc.declare_dram_parameter("yt", [P, OUT // P, ctot], f32, isOutput=True)

    with tile.TileContext(nc) as tc:
        with (
            tc.tile_pool(name="wpool", bufs=1) as wpool,
            tc.tile_pool(name="xpool", bufs=1) as xpool,
            tc.tile_pool(name="hpool", bufs=1) as hpool,
            tc.tile_pool(name="ypool", bufs=2) as ypool,
            tc.tile_pool(name="pspool", bufs=8, space="PSUM") as pspool,
        ):
            # PE warm-up: ~4.5us of dependency-free dummy matmuls so the HAM
            # clock gate reaches K=8/8 before the first real matmul's DMA
            # dependencies land, and the PE never runs throttled.
            wu = wpool.tile([P, P], bf16, tag="warmup")
            nc.gpsimd.memset(wu[:], 0.0)
            for _ in range(44):
                wps = pspool.tile([P, P], f32, tag="ps")
                nc.tensor.matmul(wps[:], wu[:], wu[:], start=True, stop=True)

            b1sb = wpool.tile([P, H // P], f32, tag="b1")
            b2sb = wpool.tile([P, OUT // P], f32, tag="b2")

            # DMA emission order is the sync-sequencer program order, which
            # sets HW-DGE FIFO order: per expert (ascending), first that
            # expert's x chunks, then the weight slices it adds on top of
            # the previous expert's nested footprint. Small experts compute
            # while the big experts' weights stream in behind them.
            #
            # Weight tiles are grouped [128, nk, cols] per (k-row range,
            # new column extent) so each group is one DMA, and an expert's
            # matmuls depend only on the groups covering slices it reads.
            w1x = {}  # k -> list of (lo, hi, k0, tile)
            w2x = {}
            xts = {}  # (e, c0) -> tile

            def _emit_wgroups(xdict, dram, nk_of, ncols_of, e, tagp):
                nk_prev = nk_of(e - 1) if e > 0 else 0
                cols_prev = ncols_of(e - 1) if e > 0 else 0
                nk, cols = nk_of(e), ncols_of(e)
                groups = []
                if nk_prev and cols > cols_prev:
                    groups.append((0, nk_prev, cols_prev, cols))
                if nk > nk_prev:
                    groups.append((nk_prev, nk, 0, cols))
                for k0, k1, lo, hi in groups:
                    t = wpool.tile([P, k1 - k0, hi - lo], bf16, tag=f"{tagp}_{k0}_{lo}")
                    nc.sync.dma_start(t[:], dram[:, k0:k1, lo:hi])
                    for k in range(k0, k1):
                        xdict.setdefault(k, []).append((lo, hi, k0, t))

            for e in range(E):
                d_in, d_hid, d_out = DIMS[e]
                nk1 = d_in // P
                cw = CHUNK_W[e]
                for c0 in range(0, caps[e], cw):
                    cn = min(cw, caps[e] - c0)
                    col = offs[e] + c0
                    xt = xpool.tile([P, nk1, cn], bf16, tag=f"xt_{e}_{c0}")
                    nc.sync.dma_start(xt[:], xt_d[:, :nk1, col : col + cn])
                    xts[(e, c0)] = xt
                    if e == 0 and c0 == 0:
                        # first matmul needs only x(e0,c0) + w1(e0); emit those
                        # first, then the cheap bias loads.
                        _emit_wgroups(w1x, w1_d, lambda i: DIMS[i][0] // P, lambda i: DIMS[i][1], 0, "w1")
                        nc.sync.dma_start(b1sb[:], b1_d[:])
                        nc.sync.dma_start(b2sb[:], b2_d[:])
                if e > 0:
                    _emit_wgroups(w1x, w1_d, lambda i: DIMS[i][0] // P, lambda i: DIMS[i][1], e, "w1")
                _emit_wgroups(w2x, w2_d, lambda i: DIMS[i][1] // P, lambda i: DIMS[i][2], e, "w2")

            def wslice(xdict, k, m):
                """[128, 128] lhsT slice for feature cols [m*128,(m+1)*128)."""
                lo_c, hi_c = m * P, (m + 1) * P
                for lo, hi, k0, t in xdict[k]:
                    if lo <= lo_c and hi_c <= hi:
                        return t[:, k - k0, lo_c - lo : hi_c - lo]
                raise AssertionError("weight slice not found")

            for e in range(E):
                d_in, d_hid, d_out = DIMS[e]
                nk1, nm1 = d_in // P, d_hid // P
                nk2, nm2 = d_hid // P, d_out // P
                cw = CHUNK_W[e]
                for c0 in range(0, caps[e], cw):
                    cn = min(cw, caps[e] - c0)
                    col = offs[e] + c0
                    xt = xts[(e, c0)]
                    ht = hpool.tile([P, nm1, cn], bf16, tag="ht")
                    for m in range(nm1):
                        ps = pspool.tile([P, cn], f32, tag="ps")
                        for k in range(nk1):
                            nc.tensor.matmul(
                                ps[:],
                                wslice(w1x, k, m),
                                xt[:, k, :],
                                start=(k == 0),
                                stop=(k == nk1 - 1),
                            )
                        nc.scalar.activation(ht[:, m, :], ps[:], Gelu, bias=b1sb[:, m : m + 1])
                    yt = ypool.tile([P, nm2, cn], f32, tag="yt")
                    for m2 in range(nm2):
                        ps = pspool.tile([P, cn], f32, tag="ps")
                        for k2 in range(nk2):
                            nc.tensor.matmul(
                                ps[:],
                                wslice(w2x, k2, m2),
                                ht[:, k2, :],
                                start=(k2 == 0),
                                stop=(k2 == nk2 - 1),
                            )
                        nc.vector.tensor_scalar_add(yt[:, m2, :], ps[:], b2sb[:, m2 : m2 + 1])
                        # stream each 128-row slab out as soon as it's ready
                        # (issued from the otherwise-idle GpSimd engine so the
                        # sync queue's weight stream is undisturbed and the
                        # kernel tail is one slab, not the whole chunk)
                        nc.gpsimd.dma_start(y_d[:, m2, col : col + cn], yt[:, m2, :])

    nc.compile()
    return nc, ctot, offs


def kernel(x, expert_mask, w1, b1, w2, b2):
    from concourse.bass_utils import run_bass_kernel_spmd

    B, N, _ = x.shape
    T = B * N
    xf = np.asarray(x, dtype=np.float32).reshape(T, D)
    mask = np.asarray(expert_mask).reshape(T).astype(np.int64)

    # --- host routing ---
    ids_by_e = [np.nonzero(mask == e)[0] for e in range(E)]
    counts = [len(i) for i in ids_by_e]
    caps = [max(64, _round_up(math.ceil(c / NCORES), 64)) for c in counts]
    # per (core, expert) token id arrays
    core_ids = [[None] * E for _ in range(NCORES)]
    for e in range(E):
        parts = np.array_split(ids_by_e[e], NCORES)
        for c in range(NCORES):
            assert len(parts[c]) <= caps[e]
            core_ids[c][e] = parts[c]

    nc, ctot, offs = _build_graph(caps)

    # --- host input prep ---
    w1t = _tile_fmajor(np.asarray(w1, np.float32).T).astype(BF16)  # [128, 8, H]
    w2t = _tile_fmajor(np.asarray(w2, np.float32).T).astype(BF16)  # [128, 32, OUT]
    b1t = np.ascontiguousarray(np.asarray(b1, np.float32).reshape(H // P, P).T)
    b2t = np.ascontiguousarray(np.asarray(b2, np.float32).reshape(OUT // P, P).T)

    in_maps = []
    for c in range(NCORES):
        xg = np.zeros((ctot, D), np.float32)
        for e in range(E):
            ids = core_ids[c][e]
            xg[offs[e] : offs[e] + len(ids)] = xf[ids]
        xt = _tile_fmajor(xg.T).astype(BF16)  # [128, 8, ctot]
        in_maps.append({"xt": xt, "w1t": w1t, "w2t": w2t, "b1t": b1t, "b2t": b2t})

    res = run_bass_kernel_spmd(nc, in_maps, list(range(NCORES)))

    # --- host output assembly ---
    y = np.zeros((T, OUT), np.float32)
    for c in range(NCORES):
        yr = np.asarray(res.results[c]["yt"])  # [128, 8, ctot]
        yfull = yr.transpose(1, 0, 2).reshape(OUT, ctot)
        for e in range(E):
            d_out = DIMS[e][2]
            ids = core_ids[c][e]
            if len(ids):
                y[ids, :d_out] = yfull[:d_out, offs[e] : offs[e] + len(ids)].T
    return y.reshape(B, N, OUT)


# revision 9
# speedup vs baseline: 1.0232x; 1.0232x over previous
"""NestedMLP MoE-routed kernel for 8 TRN2 NeuronCores.

Strategy:
  - Host routes tokens by expert (argsort of expert_mask), splits each
    expert's tokens across the 8 cores (data-parallel), pads each
    per-core expert group to a common capacity so all cores run one SPMD
    program.
  - Activations are kept feature-major ("transposed", [feature, token])
    so both matmuls are natural lhsT.T @ rhs with the contraction dim on
    partitions, and the per-feature biases are per-partition (fusable
    into the ACT/DVE PSUM eviction).
  - Weights/activations are bf16 (f32 PSUM accumulation); biases and the
    output stay f32.
  - Per expert e (shift = 3-e): d_in = 1024>>shift, d_hid = 4*d_in,
    d_out = 1024>>shift, using the nested weight slices
    w1[:d_hid,:d_in], w2[:d_out,:d_hid].
"""

import math
import sys
import types

sys.path.insert(0, "/opt/trn_rl_repo")

import ml_dtypes
import numpy as np

P = 128
E = 4
D = 1024
H = 4096
OUT = 1024
NCORES = 8
MLP_RATIO = 4

BF16 = ml_dtypes.bfloat16

# (d_in, d_hid, d_out) per expert
DIMS = [((D >> (E - 1 - e)), (D >> (E - 1 - e)) * MLP_RATIO, (OUT >> (E - 1 - e))) for e in range(E)]
# chunk width (token columns per matmul pass) per expert; expert 3 uses 256
# so its full hidden tile ([128, 32, cw] bf16) fits in SBUF next to the
# resident weights.
CHUNK_W = [512, 512, 512, 256]


def _round_up(v, m):
    return ((v + m - 1) // m) * m


def _tile_fmajor(a2d):
    """[F, C] -> [128, F//128, C] with row f = po*128 + pi."""
    f, c = a2d.shape
    return np.ascontiguousarray(a2d.reshape(f // P, P, c).transpose(1, 0, 2))


def _nested_extents(full, per_expert):
    """Column extents [(lo, hi)] that successive experts add. per_expert:
    list of high-watermarks per expert."""
    exts = []
    prev = 0
    for hi in per_expert:
        if hi > prev:
            exts.append((prev, hi))
            prev = hi
    assert prev == full
    return exts


def _build_graph(caps):
    """Build the SPMD Bass graph for per-core per-expert capacities `caps`."""
    import concourse.mybir as mybir
    import concourse.tile as tile
    from concourse import bacc

    f32 = mybir.dt.float32
    bf16 = mybir.dt.bfloat16
    Gelu = mybir.ActivationFunctionType.Gelu

    ctot = sum(caps)
    offs = np.concatenate([[0], np.cumsum(caps)]).astype(int)

    nc = bacc.Bacc(None, target_bir_lowering=False, debug=False)
    xt_d = nc.declare_dram_parameter("xt", [P, D // P, ctot], bf16, isOutput=False)
    w1_d = nc.declare_dram_parameter("w1t", [P, D // P, H], bf16, isOutput=False)
    w2_d = nc.declare_dram_parameter("w2t", [P, H // P, OUT], bf16, isOutput=False)
    b1_d = nc.declare_dram_parameter("b1t", [P, H // P], f32, isOutput=False)
    b2_d = nc.declare_dram_parameter("b2t", [P, OUT // P], f32, isOutput=False)
    y_d = nc.declare_dram_parameter("yt", [P, OUT // P, ctot], f32, isOutput=True)

    with tile.TileContext(nc) as tc:
        with (
            tc.tile_pool(name="wpool", bufs=1) as wpool,
            tc.tile_pool(name="xpool", bufs=1) as xpool,
            tc.tile_pool(name="hpool", bufs=1) as hpool,
            tc.tile_pool(name="ypool", bufs=2) as ypool,
            tc.tile_pool(name="pspool", bufs=8, space="PSUM") as pspool,
        ):
            b1sb = wpool.tile([P, H // P], f32, tag="b1")
            b2sb = wpool.tile([P, OUT // P], f32, tag="b2")

            # DMA emission order is the sync-sequencer program order, which
            # sets HW-DGE FIFO order: per expert (ascending), first that
            # expert's x chunks, then the weight slices it adds on top of
            # the previous expert's nested footprint. Small experts compute
            # while the big experts' weights stream in behind them.
            #
            # Weight tiles are grouped [128, nk, cols] per (k-row range,
            # new column extent) so each group is one DMA, and an expert's
            # matmuls depend only on the groups covering slices it reads.
            w1x = {}  # k -> list of (lo, hi, k0, tile)
            w2x = {}
            xts = {}  # (e, c0) -> tile

            def _emit_wgroups(xdict, dram, nk_of, ncols_of, e, tagp):
                nk_prev = nk_of(e - 1) if e > 0 else 0
                cols_prev = ncols_of(e - 1) if e > 0 else 0
                nk, cols = nk_of(e), ncols_of(e)
                groups = []
                if nk_prev and cols > cols_prev:
                    groups.append((0, nk_prev, cols_prev, cols))
                if nk > nk_prev:
                    groups.append((nk_prev, nk, 0, cols))
                for k0, k1, lo, hi in groups:
                    t = wpool.tile([P, k1 - k0, hi - lo], bf16, tag=f"{tagp}_{k0}_{lo}")
                    nc.sync.dma_start(t[:], dram[:, k0:k1, lo:hi])
                    for k in range(k0, k1):
                        xdict.setdefault(k, []).append((lo, hi, k0, t))

            for e in range(E):
                d_in, d_hid, d_out = DIMS[e]
                nk1 = d_in // P
                cw = CHUNK_W[e]
                for c0 in range(0, caps[e], cw):
                    cn = min(cw, caps[e] - c0)
                    col = offs[e] + c0
                    xt = xpool.tile([P, nk1, cn], bf16, tag=f"xt_{e}_{c0}")
                    nc.sync.dma_start(xt[:], xt_d[:, :nk1, col : col + cn])
                    xts[(e, c0)] = xt
                    if e == 0 and c0 == 0:
                        # first matmul needs only x(e0,c0) + w1(e0); emit those
                        # first, then the cheap bias loads.
                        _emit_wgroups(w1x, w1_d, lambda i: DIMS[i][0] // P, lambda i: DIMS[i][1], 0, "w1")
                        nc.sync.dma_start(b1sb[:], b1_d[:])
                        nc.sync.dma_start(b2sb[:], b2_d[:])
                if e > 0:
                    _emit_wgroups(w1x, w1_d, lambda i: DIMS[i][0] // P, lambda i: DIMS[i][1], e, "w1")
                _emit_wgroups(w2x, w2_d, lambda i: DIMS[i][1] // P, lambda i: DIMS[i][2], e, "w2")

            def wslice(xdict, k, m):
                """[128, 128] lhsT slice for feature cols [m*128,(m+1)*128)."""
                lo_c, hi_c = m * P, (m + 1) * P
                for lo, hi, k0, t in xdict[k]:
                    if lo <= lo_c and hi_c <= hi:
                        return t[:, k - k0, lo_c - lo : hi_c - lo]
                raise AssertionError("weight slice not found")

            for e in range(E):
                d_in, d_hid, d_out = DIMS[e]
                nk1, nm1 = d_in // P, d_hid // P
                nk2, nm2 = d_hid // P, d_out // P
                cw = CHUNK_W[e]
                for c0 in range(0, caps[e], cw):
                    cn = min(cw, caps[e] - c0)
                    col = offs[e] + c0
                    xt = xts[(e, c0)]
                    ht = hpool.tile([P, nm1, cn], bf16, tag="ht")
                    for m in range(nm1):
                        ps = pspool.tile([P, cn], f32, tag="ps")
                        for k in range(nk1):
                            nc.tensor.matmul(
                                ps[:],
                                wslice(w1x, k, m),
                                xt[:, k, :],
                                start=(k == 0),
                                stop=(k == nk1 - 1),
                            )
                        nc.scalar.activation(ht[:, m, :], ps[:], Gelu, bias=b1sb[:, m : m + 1])
                    yt = ypool.tile([P, nm2, cn], f32, tag="yt")
                    for m2 in range(nm2):
                        ps = pspool.tile([P, cn], f32, tag="ps")
                        for k2 in range(nk2):
                            nc.tensor.matmul(
                                ps[:],
                                wslice(w2x, k2, m2),
                                ht[:, k2, :],
                                start=(k2 == 0),
                                stop=(k2 == nk2 - 1),
                            )
                        nc.vector.tensor_scalar_add(yt[:, m2, :], ps[:], b2sb[:, m2 : m2 + 1])
                        # stream each 128-row slab out as soon as it's ready,
                        # so the kernel tail is one slab, not the whole chunk
                        # (these all sit after the weight DMAs in the sync
                        # stream, and the sync sequencer is idle by then)
                        nc.sync.dma_start(y_d[:, m2, col : col + cn], yt[:, m2, :])

    nc.compile()
    return nc, ctot, offs


def kernel(x, expert_mask, w1, b1, w2, b2):
    from concourse.bass_utils import run_bass_kernel_spmd

    B, N, _ = x.shape
    T = B * N
    xf = np.asarray(x, dtype=np.float32).reshape(T, D)
    mask = np.asarray(expert_mask).reshape(T).astype(np.int64)

    # --- host routing ---
    ids_by_e = [np.nonzero(mask == e)[0] for e in range(E)]
    counts = [len(i) for i in ids_by_e]
    caps = [max(64, _round_up(math.ceil(c / NCORES), 64)) for c in counts]
    # per (core, expert) token id arrays
    core_ids = [[None] * E for _ in range(NCORES)]
    for e in range(E):
        parts = np.array_split(ids_by_e[e], NCORES)
        for c in range(NCORES):
            assert len(parts[c]) <= caps[e]
            core_ids[c][e] = parts[c]

    nc, ctot, offs = _build_graph(caps)

    # --- host input prep ---
    w1t = _tile_fmajor(np.asarray(w1, np.float32).T).astype(BF16)  # [128, 8, H]
    w2t = _tile_fmajor(np.asarray(w2, np.float32).T).astype(BF16)  # [128, 32, OUT]
    b1t = np.ascontiguousarray(np.asarray(b1, np.float32).reshape(H // P, P).T)
    b2t = np.ascontiguousarray(np.asarray(b2, np.float32).reshape(OUT // P, P).T)

    in_maps = []
    for c in range(NCORES):
        xg = np.zeros((ctot, D), np.float32)
        for e in range(E):
            ids = core_ids[c][e]
            xg[offs[e] : offs[e] + len(ids)] = xf[ids]
        xt = _tile_fmajor(xg.T).astype(BF16)  # [128, 8, ctot]
        in_maps.append({"xt": xt, "w1t": w1t, "w2t": w2t, "b1t": b1t, "b2t": b2t})

    res = run_bass_kernel_spmd(nc, in_maps, list(range(NCORES)))

    # --- host output assembly ---
    y = np.zeros((T, OUT), np.float32)
    for c in range(NCORES):
        yr = np.asarray(res.results[c]["yt"])  # [128, 8, ctot]
        yfull = yr.transpose(1, 0, 2).reshape(OUT, ctot)
        for e in range(E):
            d_out = DIMS[e][2]
            ids = core_ids[c][e]
            if len(ids):
                y[ids, :d_out] = yfull[:d_out, offs[e] : offs[e] + len(ids)].T
    return y.reshape(B, N, OUT)
